# revision 52
# baseline (speedup 1.0000x reference)
"""Multi-head attention (ESIM-style masked softmax) on 8 trn2 NeuronCores.

Sharding: core c -> (batch b = c//2, head-group g = c%2). Each core runs
batch b with 8 of the 16 heads (512 channels): Wq/Wk/Wv column shards,
attention, and a partial output projection with its 512 rows of Wo. Host
sums the two partials per batch.

v2: host-side mask compaction (masked queries/keys contribute exactly 0,
so only the ~nq/nk surviving rows are shipped/computed, padded to a
multiple of 128), bf16 operands everywhere (PSUM accumulates fp32),
row-packed score matmuls (two dh=64 heads run concurrently in PE row
groups 0-1/2-3), wide exp tiles, PSUM-accumulated output projection, and
batched approx reciprocals instead of single-partition reciprocals.
"""
import sys

for _p in ("/opt/trn_rl_repo",):
    if _p not in sys.path:
        sys.path.insert(0, _p)

import numpy as np
import ml_dtypes

import concourse.bass as bass
import concourse.tile as tile
from concourse import mybir
from concourse.bass_utils import run_bass_kernel_spmd

# ---------------------------------------------------------------------------
# Workaround for this container's walrus build: it accepts at most ONE sem
# wait per lowered instruction. Split excess waits onto injected nops on the
# same (in-order) engine queue, and do the same for the kernel-tail drain.
# ---------------------------------------------------------------------------
import bass_rust
import concourse.tile as tile_mod
from concourse.vector_clock import ScopedClock

_MAX_WAITS = 1
_N_CARRIERS = 32
_wsplit_counter = [0]


def _patched_drain_and_barrier(self, tick_clock, wait_clock):
    nc = self.nc
    pre = [nc.sync.drain() for _ in range(_N_CARRIERS)]
    drain_inst = nc.sync.drain()
    wait_clock.add_sem_waits(
        drain_inst.ins, ScopedClock({None: tick_clock.global_clock})
    )
    si = drain_inst.ins.sync_info
    waits = list(si.on_wait) if si is not None else []
    if len(waits) > _MAX_WAITS:
        chunks = [waits[i : i + _MAX_WAITS] for i in range(0, len(waits), _MAX_WAITS)]
        *head, tail = chunks
        assert len(head) <= len(pre), f"too many drain waits: {len(waits)}"
        for inst, chunk in zip(pre, head):
            inst.ins.sync_info = bass_rust.SyncInfo(on_wait=chunk, on_update=[])
        drain_inst.ins.sync_info = bass_rust.SyncInfo(
            on_wait=tail, on_update=list(si.on_update) if si else []
        )
    nc.all_engine_barrier()
    assert self.sems is not None
    popped = nc._tile_sem_poison_stack.pop()
    assert popped is self._sem_poison
    nc.clear_and_free_semaphores(list(self.sems.allocated().values()))
    nc.all_engine_barrier()


def _split_excess_waits(nc, max_waits=_MAX_WAITS):
    n_split = 0
    for fn in nc.m.functions:
        for blk in fn.blocks:
            insts = blk.instructions
            if not any(
                inst.sync_info is not None
                and len(inst.sync_info.on_wait) > max_waits
                for inst in insts
            ):
                continue
            new = []
            for inst in insts:
                si = inst.sync_info
                waits = list(si.on_wait) if si is not None and si.on_wait else []
                if len(waits) > max_waits:
                    head, tail = waits[:-max_waits], waits[-max_waits:]
                    for w in head:
                        _wsplit_counter[0] += 1
                        nop = mybir.InstNoOp(
                            name=f"wsplit-{_wsplit_counter[0]}", ins=[], outs=[]
                        )
                        nop.engine = inst.engine
                        nop.sync_info = bass_rust.SyncInfo(on_wait=[w], on_update=[])
                        new.append(nop)
                        n_split += 1
                    inst.sync_info = bass_rust.SyncInfo(
                        on_wait=tail, on_update=list(si.on_update)
                    )
                new.append(inst)
            insts[:] = new
    return n_split


def _ldw_sig(inst):
    ap = inst.ins[0]
    return (
        repr(ap),
        inst.tile_position,
        inst.perf_mode,
        inst.is_transpose,
    )


def _dedup_ldweights(nc):
    """Drop an InstLdweights identical to the previous one when only
    matmuls/nops sit between them on the PE stream — the stationary
    operand is still resident in the array. Skips any candidate that
    carries semaphore waits/updates (conservative)."""
    n_drop = 0
    for fn in nc.m.functions:
        for blk in fn.blocks:
            insts = blk.instructions
            new = []
            last_sig = None
            for inst in insts:
                if isinstance(inst, mybir.InstLdweights):
                    si = inst.sync_info
                    clean = si is None or (not si.on_wait and not si.on_update)
                    sig = _ldw_sig(inst)
                    if clean and sig == last_sig:
                        n_drop += 1
                        continue
                    last_sig = sig
                elif isinstance(inst, (mybir.InstMatmult, mybir.InstNoOp)):
                    pass
                elif getattr(inst, "engine", None) == mybir.EngineType.PE:
                    last_sig = None
                new.append(inst)
            insts[:] = new
    return n_drop


_orig_tile_exit = tile_mod.TileContext.__exit__


def _patched_tile_exit(self, *args, **kwargs):
    ret = _orig_tile_exit(self, *args, **kwargs)
    _dedup_ldweights(self.nc)
    _split_excess_waits(self.nc)
    return ret


if getattr(tile_mod.TileContext, "_attn_patch", None) is None:
    tile_mod.TileContext._drain_and_barrier = _patched_drain_and_barrier
    tile_mod.TileContext.__exit__ = _patched_tile_exit
    tile_mod.TileContext._attn_patch = True

# ---------------------------------------------------------------------------
# Enable walrus's LDWEIGHTS optimization (the repo hardcodes it off; the
# serial weight-load tax is ~40us here). The NEFF cache keys on HLO content,
# so programs carry a salt to force recompiles when this changes.
# ---------------------------------------------------------------------------
from concourse import bass_utils as _bu

_orig_run_command = _bu.run_command


def _patched_run_command(cmd, *a, **kw):
    # ldw-opt=true fails walrus codegen (visitInstLdweights) on this
    # toolchain -- keep the default; hook retained for experiments.
    return _orig_run_command(cmd, *a, **kw)


if getattr(_bu, "_ldwopt_patch", None) is None:
    _bu.run_command = _patched_run_command
    _bu._ldwopt_patch = True

# ---------------------------------------------------------------------------
# Program constants
# ---------------------------------------------------------------------------
f32 = mybir.dt.float32
bf16 = mybir.dt.bfloat16
AF = mybir.ActivationFunctionType
ALU = mybir.AluOpType

B, L, D = 4, 1024, 1024
CH = 512          # channels per core (8 heads x dh=64)
DC = 8            # d (contraction) chunks of 128
N_CORES = 8
SCALE = 0.125     # 1/sqrt(dh)


def _chunks(n, rem_quantum=128):
    """Split n columns into moving-operand chunks of <=512; the final
    partial chunk is rounded up to rem_quantum."""
    out = []
    off = 0
    while off < n:
        w = min(512, n - off)
        if w < 512:
            w = -(-w // rem_quantum) * rem_quantum
        out.append((off, w))
        off += w
    return out


SALT = 6  # bumped to bust the content-keyed NEFF cache on flag changes


def build_program(NQ, NK, nqv=None):
    NQC, NKC = NQ // 128, NK // 128
    if nqv is None:
        nqv = NQ
    nc = bass.Bass(trn_type="TRN2", target_bir_lowering=False, debug=False)

    qT_d = nc.dram_tensor("qT", [D, NQ], bf16, kind="ExternalInput").ap()
    kT_d = nc.dram_tensor("kT", [D, NK], bf16, kind="ExternalInput").ap()
    vT_d = nc.dram_tensor("vT", [D, NK], bf16, kind="ExternalInput").ap()
    wq_d = nc.dram_tensor("wq", [D, CH], bf16, kind="ExternalInput").ap()
    wk_d = nc.dram_tensor("wk", [D, CH], bf16, kind="ExternalInput").ap()
    wv_d = nc.dram_tensor("wv", [D, CH], bf16, kind="ExternalInput").ap()
    wo_d = nc.dram_tensor("wo", [CH, D], bf16, kind="ExternalInput").ap()
    km_d = nc.dram_tensor("km", [128, NKC], bf16, kind="ExternalInput").ap()
    out_d = nc.dram_tensor(f"out_s{SALT}", [NQ, D], bf16,
                           kind="ExternalOutput").ap()

    qcl = _chunks(nqv, rem_quantum=32)

    with tile.TileContext(nc) as tc:
        with (
            tc.tile_pool(name="persist", bufs=1) as pers,
            tc.tile_pool(name="work", bufs=3) as work,
        ):
            # ---- persistent SBUF tiles ----
            km_t = pers.tile([128, NKC], bf16, tag="km")
            wo_t = pers.tile([128, 4 * 1024], bf16, tag="wo")
            QT_t = pers.tile([128, 4 * NQ], bf16, tag="QT")
            KT_t = pers.tile([128, 4 * NK], bf16, tag="KT")
            # per (ki, head): 128 cols = [64 V-cols | 64 replicated km], so
            # each head's PV matmul computes numerator on partitions 0:64
            # and broadcasts the softmax denominator onto 64:128 in the
            # same full-density pass.
            V_t = pers.tile([128, NKC * 1024], bf16, tag="V")
            OT_ts = [pers.tile([128, NQ], bf16, tag=f"OT{i}", name=f"OT{i}")
                     for i in range(4)]
            # staging for full kT/qT/vT (bf16, 8 d-chunks each)
            k_sb = pers.tile([128, DC * NK], bf16, tag="k_sb")
            q_sb = pers.tile([128, DC * NQ], bf16, tag="q_sb")
            v_sb = pers.tile([128, DC * NK], bf16, tag="v_sb")

            # ---- input DMAs: chunked per d-slice so compute starts as
            # soon as the first slices land; spread across 4 queues ----
            k3 = kT_d.rearrange("(d p) l -> p d l", p=128)
            q3 = qT_d.rearrange("(d p) l -> p d l", p=128)
            v3 = vT_d.rearrange("(d p) l -> p d l", p=128)

            def load_w(pool, dram, eng):
                t = pool.tile([128, DC * 512], bf16, tag=dram.tensor.name + "_t")
                s3 = dram.rearrange("(d p) n -> p d n", p=128)
                eng.dma_start(t[:].rearrange("p (d n) -> p d n", d=DC), s3)
                return t

            with tc.tile_pool(name="wpool", bufs=1) as wpool:
                # need-order loads, split per d-slice and round-robined over
                # the three DMA-issuing queues so the first V-projection
                # matmul can start after ~0.3MB instead of ~2.3MB.
                engs = (nc.sync, nc.scalar, nc.gpsimd)
                ecnt = [0]

                def qdma(dst, src):
                    engs[ecnt[0] % 3].dma_start(dst, src)
                    ecnt[0] += 1

                def load_w_chunked(dram):
                    t = wpool.tile([128, DC * 512], bf16,
                                   tag=dram.tensor.name + "_t",
                                   name=dram.tensor.name + "_t")
                    s3 = dram.rearrange("(d p) n -> p d n", p=128)
                    return t, s3

                wv_t, wv3 = load_w_chunked(wv_d)
                wk_t, wk3 = load_w_chunked(wk_d)
                wq_t, wq3 = load_w_chunked(wq_d)
                nc.scalar.dma_start(km_t[:], km_d)
                for d in range(DC):
                    qdma(wv_t[:, d * 512:(d + 1) * 512], wv3[:, d])
                    qdma(v_sb[:, d * NK:(d + 1) * NK], v3[:, d])
                for d in range(DC):
                    qdma(wk_t[:, d * 512:(d + 1) * 512], wk3[:, d])
                    qdma(k_sb[:, d * NK:(d + 1) * NK], k3[:, d])
                for d in range(DC):
                    qdma(wq_t[:, d * 512:(d + 1) * 512], wq3[:, d])
                    qdma(q_sb[:, d * NQ:(d + 1) * NQ], q3[:, d])
                wo3 = wo_d.rearrange("(c p) n -> p c n", p=128)
                nc.gpsimd.dma_start(
                    wo_t[:].rearrange("p (c n) -> p c n", c=4), wo3)

                # ---- V projection: d-outer (starts on first vT slice),
                # NKC psum accumulators ----
                with tc.tile_pool(name="psV", bufs=NKC, space="PSUM") as psV:
                    psv = [psV.tile([128, 512], f32, tag="psv",
                                    name=f"psv{ki}") for ki in range(NKC)]
                    for d in range(DC):
                        for ki in range(NKC):
                            nc.tensor.matmul(
                                psv[ki][:],
                                v_sb[:, d * NK + ki * 128: d * NK + (ki + 1) * 128],
                                wv_t[:, d * 512:(d + 1) * 512],
                                start=(d == 0), stop=(d == DC - 1),
                            )
                    for ki in range(NKC):
                        blk = V_t[:, ki * 1024:(ki + 1) * 1024].rearrange(
                            "p (h c) -> p h c", c=128)
                        nc.vector.tensor_copy(
                            blk[:, :, 0:64],
                            psv[ki][:].rearrange("p (h c) -> p h c", c=64),
                        )
                        nc.vector.tensor_copy(
                            blk[:, :, 64:128],
                            km_t[:, ki:ki + 1][:, None, :].to_broadcast(
                                (128, 8, 64)),
                        )

                # ---- K projection: d-outer (starts on first kT slice) ----
                kcl = _chunks(NK)
                with tc.tile_pool(name="psK", bufs=8, space="PSUM") as psK:
                    psk = {}
                    for ci in range(4):
                        for off, w in kcl:
                            psk[(ci, off)] = psK.tile(
                                [128, 512], f32, tag="psk",
                                name=f"psk_{ci}_{off}")
                    for d in range(DC):
                        for ci in range(4):
                            for off, w in kcl:
                                nc.tensor.matmul(
                                    psk[(ci, off)][:, 0:w],
                                    wk_t[:, d * 512 + ci * 128:
                                         d * 512 + (ci + 1) * 128],
                                    k_sb[:, d * NK + off: d * NK + off + w],
                                    start=(d == 0), stop=(d == DC - 1),
                                )
                    for ci in range(4):
                        for off, w in kcl:
                            nc.vector.tensor_copy(
                                KT_t[:, ci * NK + off: ci * NK + off + w],
                                psk[(ci, off)][:, 0:w],
                            )

                # ---- Q projection (ci-outer) interleaved with attention:
                # pair p's scores+exp start right after Q(ci=p) ----
                KI2 = (NKC + 1) // 2
                pv_backlog = []   # deferred PV+normalize emitters

                with (
                    tc.tile_pool(name="psP", bufs=2, space="PSUM") as psP,
                    tc.tile_pool(name="psST", bufs=1, space="PSUM") as psST,
                    tc.tile_pool(name="psU", bufs=2, space="PSUM") as psU,
                ):
                    def emit_qproj(ci):
                        # d-outer with chunk-inner so consecutive matmuls
                        # share the wq stationary (dedup drops the reload)
                        pst = {}
                        for off, w in qcl:
                            pst[off] = psP.tile([128, 512], f32, tag="ps",
                                                name=f"psq_{ci}_{off}")
                        for d in range(DC):
                            for off, w in qcl:
                                nc.tensor.matmul(
                                    pst[off][:, 0:w],
                                    wq_t[:, d * 512 + ci * 128:
                                         d * 512 + (ci + 1) * 128],
                                    q_sb[:, d * NQ + off: d * NQ + off + w],
                                    start=(d == 0), stop=(d == DC - 1),
                                )
                        for off, w in qcl:
                            nc.vector.tensor_copy(
                                QT_t[:, ci * NQ + off: ci * NQ + off + w],
                                pst[off][:, 0:w],
                            )

                    def emit_attn(p, ciq):
                        qoff, qN = qcl[ciq]
                        co = p * NK   # KT col offset for this pair
                        et_tiles = []
                        for ki in range(NKC):
                            # fill the exp-lag with ready deferred PV work
                            if pv_backlog:
                                pv_backlog.pop(0)()
                            # 2 bank-aligned slots (one per head) per ki
                            st = psST.tile([128, 1024], f32, tag="st",
                                           name=f"st_{p}_{ciq}_{ki}")
                            ksl = slice(co + ki * 128, co + (ki + 1) * 128)
                            for hh in range(2):
                                rows = slice(hh * 64, (hh + 1) * 64)
                                nc.tensor.matmul(
                                    st[:, hh * 512: hh * 512 + qN],
                                    KT_t[rows, ksl],
                                    QT_t[rows, p * NQ + qoff:
                                         p * NQ + qoff + qN],
                                    start=True, stop=True,
                                )
                            et = work.tile([128, 1024], bf16, tag="et",
                                           name=f"et_{p}_{ciq}_{ki}", bufs=16)
                            st3 = st[:].rearrange("p (s c) -> p s c", c=512)
                            et3 = et[:].rearrange("p (s c) -> p s c", c=qN)
                            nc.scalar.activation(
                                et3[:, 0:2, :], st3[:, 0:2, 0:qN],
                                AF.Exp, scale=SCALE,
                            )
                            et_tiles.append(et)
                        pv_backlog.extend(_mk_pv(p, ciq, qoff, qN, et_tiles))

                    def _mk_pv(p, ciq, qoff, qN, et_tiles):
                        # Sliced into per-ki micro-emitters (2 matmuls each)
                        # popped between score matmuls of later chunks, so
                        # the PE fills exp-lag stalls with ready PV work.
                        # One [128,1024] u tile per (pair, chunk): head A at
                        # cols 0:512, head B at 512:1024; each head's rows
                        # 64:128 carry the PE-broadcast denominator.
                        tiles = {}

                        def pv_ki(ki):
                            def emit():
                                if not tiles:
                                    tiles["u"] = psU.tile(
                                        [128, 1024], f32, tag="u",
                                        name=f"u_{p}_{ciq}")
                                u = tiles["u"]
                                et = et_tiles[ki]
                                for hh in range(2):
                                    h = 2 * p + hh
                                    nc.tensor.matmul(
                                        u[:, hh * 512: hh * 512 + qN],
                                        V_t[:, ki * 1024 + h * 128:
                                            ki * 1024 + (h + 1) * 128],
                                        et[:, hh * qN:(hh + 1) * qN],
                                        start=(ki == 0),
                                        stop=(ki == NKC - 1),
                                    )
                            return emit

                        def norm():
                            u = tiles["u"]
                            rc = work.tile([128, 1024], f32, tag="rc",
                                           name=f"rc_{p}_{ciq}")
                            # head A denominator -> DVE reciprocal; head B
                            # -> ACT exp(-ln(x)) (same table set as the
                            # score exp). The two run on different engines
                            # in parallel, halving normalize latency.
                            nc.vector.reciprocal(
                                rc[64:128, 0:qN], u[64:128, 0:qN])
                            lt = work.tile([128, 1024], f32, tag="lt",
                                           name=f"lt_{p}_{ciq}")
                            nc.scalar.activation(
                                lt[64:128, 512:512 + qN],
                                u[64:128, 512:512 + qN], AF.Ln)
                            nc.scalar.activation(
                                rc[64:128, 512:512 + qN],
                                lt[64:128, 512:512 + qN], AF.Exp,
                                scale=-1.0)
                            for hh in range(2):
                                nc.vector.tensor_tensor(
                                    OT_ts[p][hh * 64:(hh + 1) * 64,
                                             qoff:qoff + qN],
                                    u[0:64, hh * 512: hh * 512 + qN],
                                    rc[64:128, hh * 512: hh * 512 + qN],
                                    ALU.mult,
                                )

                        def last():
                            pv_ki(NKC - 1)()
                            norm()

                        return [pv_ki(ki) for ki in range(NKC - 1)] + [last]

                    for ci in range(4):
                        emit_qproj(ci)
                        for ciq in range(len(qcl)):
                            emit_attn(ci, ciq)
                    while pv_backlog:
                        pv_backlog.pop(0)()

            # ---- output projection: accumulate over head pairs in PSUM;
            # hp-outer/oh-inner so the OT stationary is shared by the two
            # oh matmuls (dedup drops the reload) ----
            with tc.tile_pool(name="psO", bufs=4, space="PSUM") as psO:
                for li in range(NQC):
                    po = [psO.tile([128, 512], f32, tag="po",
                                   name=f"po_{li}_{oh}") for oh in range(2)]
                    for hp in range(4):
                        for oh in range(2):
                            nc.tensor.matmul(
                                po[oh][:],
                                OT_ts[hp][:, li * 128:(li + 1) * 128],
                                wo_t[:, hp * 1024 + oh * 512:
                                     hp * 1024 + (oh + 1) * 512],
                                start=(hp == 0), stop=(hp == 3),
                            )
                    for oh in range(2):
                        ob = work.tile([128, 512], bf16, tag="ob",
                                       name=f"ob_{li}_{oh}")
                        (nc.scalar.copy if oh == 0
                         else nc.vector.tensor_copy)(ob[:], po[oh][:])
                        (nc.sync, nc.gpsimd)[oh].dma_start(
                            out_d[li * 128:(li + 1) * 128,
                                  oh * 512:(oh + 1) * 512],
                            ob[:],
                        )
    return nc


_cache = {}


def _get_program(NQ, NK, nqv=None):
    key = (NQ, NK, nqv)
    if key not in _cache:
        _cache[key] = build_program(NQ, NK, nqv)
    return _cache[key]


def _pad_cap(n):
    return max(128, -(-n // 128) * 128)


def kernel(query, key, value, query_mask, key_mask, Wq, Wk, Wv, Wo,
           _trace=False):
    query = np.asarray(query, dtype=np.float32)
    key = np.asarray(key, dtype=np.float32)
    value = np.asarray(value, dtype=np.float32)
    query_mask = np.asarray(query_mask)
    key_mask = np.asarray(key_mask)
    Wq = np.asarray(Wq, dtype=np.float32)
    Wk = np.asarray(Wk, dtype=np.float32)
    Wv = np.asarray(Wv, dtype=np.float32)
    Wo = np.asarray(Wo, dtype=np.float32)

    qidx = [np.nonzero(query_mask[b])[0] for b in range(B)]
    kidx = [np.nonzero(key_mask[b])[0] for b in range(B)]
    nqv = max(len(ix) for ix in qidx)
    NQ = _pad_cap(nqv)
    NK = _pad_cap(max(len(ix) for ix in kidx))
    NKC = NK // 128

    nc = _get_program(NQ, NK, nqv)

    bf = ml_dtypes.bfloat16
    wq_g = [np.ascontiguousarray(Wq[:, g * CH:(g + 1) * CH]).astype(bf)
            for g in range(2)]
    wk_g = [np.ascontiguousarray(Wk[:, g * CH:(g + 1) * CH]).astype(bf)
            for g in range(2)]
    wv_g = [np.ascontiguousarray(Wv[:, g * CH:(g + 1) * CH]).astype(bf)
            for g in range(2)]
    wo_g = [np.ascontiguousarray(Wo[g * CH:(g + 1) * CH, :]).astype(bf)
            for g in range(2)]

    qT, kT, vT, km = [], [], [], []
    for b in range(B):
        qc = np.zeros((D, NQ), dtype=bf)
        qc[:, :len(qidx[b])] = query[b][qidx[b]].T
        qT.append(qc)
        kc = np.zeros((D, NK), dtype=bf)
        kc[:, :len(kidx[b])] = key[b][kidx[b]].T
        kT.append(kc)
        vc = np.zeros((D, NK), dtype=bf)
        vc[:, :len(kidx[b])] = value[b][kidx[b]].T
        vT.append(vc)
        kmv = np.zeros(NK, dtype=np.float32)
        kmv[:len(kidx[b])] = 1.0
        km.append(np.ascontiguousarray(
            kmv.reshape(NKC, 128).T.astype(bf)))

    in_maps = []
    for c in range(N_CORES):
        b, g = c // 2, c % 2
        in_maps.append({
            "qT": qT[b], "kT": kT[b], "vT": vT[b],
            "wq": wq_g[g], "wk": wk_g[g], "wv": wv_g[g], "wo": wo_g[g],
            "km": km[b],
        })

    res = run_bass_kernel_spmd(nc, in_maps, list(range(N_CORES)),
                               trace=_trace)
    out = np.zeros((B, L, D), dtype=np.float32)
    for b in range(B):
        okey = f"out_s{SALT}"
        part = (res.results[2 * b][okey].astype(np.float32)
                + res.results[2 * b + 1][okey].astype(np.float32))
        out[b][qidx[b]] = part[:len(qidx[b])]
    if _trace:
        return out, res
    return out
